# revision 25
# baseline (speedup 1.0000x reference)
"""Bidirectional cross-attention kernel for 8 Trainium2 NeuronCores.

Sharding: core c = 2*b + g handles batch b with head-group g (8 of 16 heads).
Each core projects Q/K/V/CV for its 8 heads (full sequence), computes both
softmax orientations of the shared similarity matrix, and forms the per-head
attention outputs U = attn @ cv and W = context_attn^T @ v (stored transposed,
pre-scaled by the softmax normalizers).  The two cores of a batch exchange
their U/W halves with a pairwise AllGather, after which each core computes a
disjoint 512-column slice of both final projections.  The host gather is pure
concatenation.

Layout notes:
 - x[b] / context[b] are fed pre-transposed ([dim, seq]) so every matmul sees
   its contraction dim on partitions; no on-device transposes are needed.
 - fp32 data is declared float32r so the PE runs at 1 cycle/row.
 - exp(sim) is stored in bf16; V/CV are stored in bf16 with an extra ones
   column per head so the PE produces the softmax normalizers for free.
 - w_out / cw_out are fed with rows pre-reordered to match the AllGather
   output layout, and column-sliced per core.
"""

import os
import sys

import numpy as np

for _p in ("/opt/trn_rl_repo", "/root/.axon_site/_ro/trn_rl_repo"):
    if os.path.isdir(_p) and _p not in sys.path:
        sys.path.append(_p)

import ml_dtypes  # noqa: E402
import concourse.bass as bass  # noqa: E402
import concourse.mybir as mybir  # noqa: E402
import concourse.tile as tile  # noqa: E402
from concourse import bacc  # noqa: E402
from concourse.bass_utils import run_bass_kernel_spmd  # noqa: E402

B, N, DIM = 4, 1024, 1024
H, DH = 16, 64
HL = 8            # heads per core
IL = HL * DH      # local inner width (512)
COLS = 512        # output columns per core
P = 128
PAIRS = HL // 2   # head pairs per core
KCH = DIM // P    # contraction chunks (8)
ICH = N // P      # sequence chunks (8)
SCALE = DH ** -0.5
GROUPS = [[0, 1], [2, 3], [4, 5], [6, 7]]

F32 = mybir.dt.float32
F32R = mybir.dt.float32r
BF16 = mybir.dt.bfloat16
EXP = mybir.ActivationFunctionType.Exp

_CACHED_NC = None


def _build_nc():
    nc = bacc.Bacc("TRN2", target_bir_lowering=False, debug=False, num_devices=8)

    xT = nc.dram_tensor("xT", [DIM, N], BF16, kind="ExternalInput")
    ctxT = nc.dram_tensor("ctxT", [DIM, N], BF16, kind="ExternalInput")
    wqk = nc.dram_tensor("wqk", [DIM, IL], BF16, kind="ExternalInput")
    wv = nc.dram_tensor("wv", [DIM, IL], BF16, kind="ExternalInput")
    cwqk = nc.dram_tensor("cwqk", [DIM, IL], BF16, kind="ExternalInput")
    cwv = nc.dram_tensor("cwv", [DIM, IL], BF16, kind="ExternalInput")
    wout = nc.dram_tensor("wout", [DIM, COLS], BF16, kind="ExternalInput")
    cwout = nc.dram_tensor("cwout", [DIM, COLS], BF16, kind="ExternalInput")
    bout = nc.dram_tensor("bout", [1, COLS], F32, kind="ExternalInput")
    cbout = nc.dram_tensor("cbout", [1, COLS], F32, kind="ExternalInput")

    out_cols = nc.dram_tensor("out_cols", [N, COLS], F32, kind="ExternalOutput")
    ctx_cols = nc.dram_tensor("ctx_cols", [N, COLS], F32, kind="ExternalOutput")

    with tile.TileContext(nc) as tc:
        # DRAM scratch (tracked by Tile)
        with tc.tile_pool(name="dram", bufs=1, space="DRAM") as dpool:
            uwl = [dpool.tile([256, N], BF16, tag=f"uwl{p}", name=f"uwl{p}")
                   for p in range(4)]
            uwa = [dpool.tile([512, N], BF16, tag=f"uwa{p}", name=f"uwa{p}")
                   for p in range(3)]
            uwa3u = dpool.tile([256, N], BF16, tag="uwa3u")
            uwa3w = dpool.tile([256, N], BF16, tag="uwa3w")
            normd = dpool.tile([16, N], F32, tag="normd")
            _build_body(nc, tc, dict(
                xT=xT, ctxT=ctxT, wqk=wqk, wv=wv, cwqk=cwqk, cwv=cwv,
                wout=wout, cwout=cwout, bout=bout, cbout=cbout,
                out_cols=out_cols, ctx_cols=ctx_cols,
                uwl=uwl, uwa=uwa, uwa3u=uwa3u, uwa3w=uwa3w, normd=normd,
            ))
    nc.compile()
    _dedupe_ldweights(nc)
    return nc


def _dedupe_ldweights(nc):
    """Drop PE Ldweights that reload the exact weights already resident.

    bass emits one Ldweights per matmul; back-to-back matmuls that share a
    stationary operand (our N-half pairs) reload it redundantly, and the
    walrus pass that would elide these (--enable-ldw-opt) rejects this
    program.  The PE keeps its stationary operand across matmuls, so a
    repeat load with no attached semaphore activity can be removed.
    """
    def sig(i):
        a = i.ins[0]
        return (a.memref, a.offset, str(a.ap), str(a.dtype),
                str(i.tile_position), str(i.tile_size),
                str(i.perf_mode), str(i.is_transpose))

    removed = 0
    for fn in nc.m.functions:
        for bb in fn.blocks:
            last = None
            keep = []
            for i in bb.instructions:
                if isinstance(i, mybir.InstLdweights):
                    s = sig(i)
                    si = i.sync_info
                    if s == last and (si is None or
                                      (not si.on_wait and not si.on_update)):
                        removed += 1
                        continue
                    last = s
                elif isinstance(i, mybir.InstMatmult):
                    pass  # does not disturb the loaded weights
                elif getattr(i, "engine", None) == mybir.EngineType.PE:
                    last = None
                keep.append(i)
            if removed:
                bb.instructions = keep
    return removed


def _build_body(nc, tc, T):
    hs = 65  # head stride in the V/CV tiles (64 values + ones column)

    from contextlib import ExitStack
    stack = ExitStack()
    pqk = stack.enter_context(tc.tile_pool(name="pqk", bufs=1))
    pv = stack.enter_context(tc.tile_pool(name="pv", bufs=1))
    pf = stack.enter_context(tc.tile_pool(name="pf", bufs=1))
    with tc.tile_pool(name="pin", bufs=1) as pin, \
         tc.tile_pool(name="pw", bufs=12) as pw, \
         tc.tile_pool(name="psA", bufs=4, space="PSUM") as psA:

        # ---- Phase A: load transposed activations, project Q/K/V/CV ----
        def load_w(name, eng, tag):
            tiles = []
            for k in range(KCH):
                t = pw.tile([P, IL], BF16, tag=tag, name=f"{name}{k}")
                eng.dma_start(t[:], T[name][k * P:(k + 1) * P, :])
                tiles.append(t)
            return tiles

        # critical-path first: wqk + x on the sync queue, context side on the
        # scalar queue; output-projection weights are loaded at the end.
        xt = []
        ct = []
        t = pin.tile([P, N], BF16, tag="xT0")
        nc.sync.dma_start(t[:], T["xT"][0:P, :])
        xt.append(t)
        wqk_t = load_w("wqk", nc.sync, "w")
        for k in range(1, KCH):
            t = pin.tile([P, N], BF16, tag=f"xT{k}", name=f"xt{k}")
            nc.sync.dma_start(t[:], T["xT"][k * P:(k + 1) * P, :])
            xt.append(t)
        cwqk_t = load_w("cwqk", nc.scalar, "w2")
        for k in range(KCH):
            t = pin.tile([P, N], BF16, tag=f"cT{k}")
            nc.scalar.dma_start(t[:], T["ctxT"][k * P:(k + 1) * P, :])
            ct.append(t)

        def proj_T(src, wtiles, out_tag):
            """Per head-pair m: two zero-padded [128, N] tiles (head A rows
            0:64 / head B rows 64:128, rest zeros) so sim matmuls run with a
            full K=128 contraction (keeps the PE HAM-warm, enables FWL)."""
            outs = []
            pss = [psA.tile([P, N], F32, tag="pa", name=f"pt_{out_tag}{m}")
                   for m in range(IL // P)]
            for k in range(KCH):
                for m in range(IL // P):
                    lhsT = wtiles[k][:, m * P:(m + 1) * P]
                    nc.tensor.matmul(pss[m][:, 0:512], lhsT, src[k][:, 0:512],
                                     start=(k == 0), stop=(k == KCH - 1))
                    nc.tensor.matmul(pss[m][:, 512:1024], lhsT, src[k][:, 512:1024],
                                     start=(k == 0), stop=(k == KCH - 1))
            for m in range(IL // P):
                ps = pss[m]
                pa = pqk.tile([P, N], BF16, tag=f"{out_tag}a{m}")
                nc.vector.tensor_copy(pa[0:DH, :], ps[0:DH, :])
                nc.vector.memset(pa[DH:P, :], 0.0)
                pb = pqk.tile([P, N], BF16, tag=f"{out_tag}b{m}")
                nc.vector.memset(pb[0:DH, :], 0.0)
                nc.vector.tensor_copy(pb[DH:P, :], ps[DH:P, :])
                outs.append((pa, pb))
            return outs

        def proj_V(src, wtiles, out_tag):
            """out[ic] [128, 8*65] bf16 = src-rows @ w, head-strided + ones."""
            outs = []
            for ic in range(ICH):
                psf = psA.tile([P, N], F32, tag="pa", name=f"pav_{out_tag}{ic}")
                ps = psf[:, 0:IL]
                for k in range(KCH):
                    nc.tensor.matmul(ps[:], src[k][:, ic * P:(ic + 1) * P],
                                     wtiles[k][:],
                                     start=(k == 0), stop=(k == KCH - 1))
                o = pv.tile([P, HL * hs], BF16, tag=f"{out_tag}{ic}")
                dst = o[:].rearrange("p (h e) -> p h e", e=hs)
                nc.vector.tensor_copy(dst[:, :, 0:DH],
                                      ps[:].rearrange("p (h e) -> p h e", e=DH))
                nc.vector.memset(dst[:, :, DH:hs], 1.0)
                outs.append(o)
            return outs

        QT = proj_T(xt, wqk_t, "qt")
        wv_t = load_w("wv", nc.sync, "w")
        KT = proj_T(ct, cwqk_t, "kt")
        cwv_t = load_w("cwv", nc.scalar, "w2")
        V = proj_V(xt, wv_t, "v")
        CV = proj_V(ct, cwv_t, "cv")

        # output-side weights/biases (needed only in the final phase)
        bout_bc = pf.tile([P, COLS], F32, tag="bb")
        nc.scalar.dma_start(bout_bc[:], T["bout"][:].to_broadcast((P, COLS)))
        cbout_bc = pf.tile([P, COLS], F32, tag="cbb")
        nc.scalar.dma_start(cbout_bc[:], T["cbout"][:].to_broadcast((P, COLS)))
        wout_sb = []
        cwout_sb = []
        for k in range(KCH):
            t = pf.tile([P, COLS], BF16, tag=f"wo{k}")
            nc.scalar.dma_start(t[:], T["wout"][k * P:(k + 1) * P, :])
            wout_sb.append(t)
            t = pf.tile([P, COLS], BF16, tag=f"cwo{k}")
            nc.scalar.dma_start(t[:], T["cwout"][k * P:(k + 1) * P, :])
            cwout_sb.append(t)

    # ---- Phase B: per head-pair attention ----
    pu = stack.enter_context(tc.tile_pool(name="pu", bufs=1))
    u_sb = [None] * KCH
    w_sb = [None] * KCH

    with tc.tile_pool(name="pe", bufs=10) as pe, \
         tc.tile_pool(name="pn", bufs=2) as pn, \
         tc.tile_pool(name="psB", bufs=2, space="PSUM") as psB:

        def load_uw(k, src_tile, u_off, w_off):
            usrc = src_tile if src_tile is not None else T["uwa3u"]
            wsrc = src_tile if src_tile is not None else T["uwa3w"]
            t = pu.tile([P, N], BF16, tag=f"ua{k}")
            nc.sync.dma_start(t[:], usrc[u_off:u_off + P, :])
            u_sb[k] = t
            t = pu.tile([P, N], BF16, tag=f"wa{k}")
            nc.sync.dma_start(t[:], wsrc[w_off:w_off + P, :])
            w_sb[k] = t

        for p in range(PAIRS):
            E = [[None] * ICH, [None] * ICH]
            ET = [[None] * ICH, [None] * ICH]

            def norm_store(psum, slot, dst, dst_row):
                """scale rows 0:64 of psum by 1/row64 (per free element), store."""
                rst = pn.tile([DH + 1, N], F32, tag="rst")
                nc.vector.tensor_copy(rst[:], psum[0:DH + 1, :])
                nc.sync.dma_start(T["normd"][slot:slot + 1, :], rst[DH:DH + 1, :])
                rbc = pn.tile([DH, N], F32, tag="rbc")
                nc.sync.dma_start(
                    rbc[:], T["normd"][slot:slot + 1, :].to_broadcast((DH, N)))
                nc.vector.reciprocal_approx_fast(rbc[:], rbc[:])
                ubf = pn.tile([DH, N], BF16, tag="ubf")
                nc.vector.tensor_mul(ubf[:], rst[0:DH, :], rbc[:])
                nc.sync.dma_start(dst[dst_row:dst_row + DH, :], ubf[:])

            # --- simT -> ET, with U-accumulation laddered in (lag 2) ---
            ups = [psB.tile([P, N], F32, tag="uw", name=f"ups{p}_{hh}")
                   for hh in range(2)]

            def u_step(hh, jc):
                h = 2 * p + hh
                lhsT = CV[jc][:, h * hs:(h + 1) * hs]
                nc.tensor.matmul(ups[hh][0:hs, 0:512], lhsT, ET[hh][jc][:, 0:512],
                                 start=(jc == 0), stop=(jc == ICH - 1))
                nc.tensor.matmul(ups[hh][0:hs, 512:1024], lhsT,
                                 ET[hh][jc][:, 512:1024],
                                 start=(jc == 0), stop=(jc == ICH - 1))

            for jc in range(ICH):
                for hh in range(2):
                    part = slice(hh * DH, (hh + 1) * DH)
                    ps = psB.tile([P, N], F32, tag="sim")
                    lhsT = KT[p][hh][part, jc * P:(jc + 1) * P]
                    nc.tensor.matmul(ps[:, 0:512], lhsT, QT[p][hh][part, 0:512],
                                     start=True, stop=True)
                    nc.tensor.matmul(ps[:, 512:1024], lhsT, QT[p][hh][part, 512:1024],
                                     start=True, stop=True)
                    e = pe.tile([P, N], BF16, tag="ET")
                    nc.scalar.activation(e[:], ps[:], EXP, scale=SCALE)
                    ET[hh][jc] = e
                if jc >= 2:
                    for hh in range(2):
                        u_step(hh, jc - 2)
            for jc in (ICH - 2, ICH - 1):
                for hh in range(2):
                    u_step(hh, jc)
            for hh in range(2):
                norm_store(ups[hh], p * 4 + hh, T["uwl"][p], hh * DH)

            if p == 3:
                nc.gpsimd.collective_compute(
                    "AllGather", mybir.AluOpType.bypass,
                    replica_groups=GROUPS,
                    ins=[T["uwl"][3][0:128, :]],
                    outs=[T["uwa3u"][:]],
                )

            # --- sim -> E, with W-accumulation laddered in (lag 2) ---
            wps = [psB.tile([P, N], F32, tag="uw", name=f"wps{p}_{hh}")
                   for hh in range(2)]

            def w_step(hh, ic):
                h = 2 * p + hh
                lhsT = V[ic][:, h * hs:(h + 1) * hs]
                nc.tensor.matmul(wps[hh][0:hs, 0:512], lhsT, E[hh][ic][:, 0:512],
                                 start=(ic == 0), stop=(ic == ICH - 1))
                nc.tensor.matmul(wps[hh][0:hs, 512:1024], lhsT,
                                 E[hh][ic][:, 512:1024],
                                 start=(ic == 0), stop=(ic == ICH - 1))

            for ic in range(ICH):
                for hh in range(2):
                    part = slice(hh * DH, (hh + 1) * DH)
                    ps = psB.tile([P, N], F32, tag="sim")
                    lhsT = QT[p][hh][part, ic * P:(ic + 1) * P]
                    nc.tensor.matmul(ps[:, 0:512], lhsT, KT[p][hh][part, 0:512],
                                     start=True, stop=True)
                    nc.tensor.matmul(ps[:, 512:1024], lhsT, KT[p][hh][part, 512:1024],
                                     start=True, stop=True)
                    e = pe.tile([P, N], BF16, tag="E")
                    nc.scalar.activation(e[:], ps[:], EXP, scale=SCALE)
                    E[hh][ic] = e
                if ic >= 2:
                    for hh in range(2):
                        w_step(hh, ic - 2)
            for ic in (ICH - 2, ICH - 1):
                for hh in range(2):
                    w_step(hh, ic)
            for hh in range(2):
                norm_store(wps[hh], p * 4 + 2 + hh, T["uwl"][p], 128 + hh * DH)

            # exchange this pair's U/W halves within the batch pair.
            # Pair 3 is split so its U half can ship before W finishes.
            if p < 3:
                nc.gpsimd.collective_compute(
                    "AllGather", mybir.AluOpType.bypass,
                    replica_groups=GROUPS,
                    ins=[T["uwl"][p][:]],
                    outs=[T["uwa"][p][:]],
                )
                load_uw(2 * p, T["uwa"][p], 0, 128)
                load_uw(2 * p + 1, T["uwa"][p], 256, 384)
            else:
                nc.gpsimd.collective_compute(
                    "AllGather", mybir.AluOpType.bypass,
                    replica_groups=GROUPS,
                    ins=[T["uwl"][3][128:256, :]],
                    outs=[T["uwa3w"][:]],
                )
                load_uw(6, None, 0, 0)
                load_uw(7, None, 128, 128)

    # ---- Phase C/D: load gathered U/W, final projections ----
    with tc.tile_pool(name="po", bufs=3) as po, \
         tc.tile_pool(name="psD", bufs=6, space="PSUM") as psD:

        # out-projection (needs all U chunks; U arrives before W), with the
        # ctx-projection's early chunks (pairs 0-2) interleaved so the PE has
        # work while pair 3's W AllGather is still in flight.
        ctx_part = []
        for ic in range(ICH):
            ps = psD.tile([P, COLS], F32, tag="od")
            for k in range(KCH):
                nc.tensor.matmul(ps[:], u_sb[k][:, ic * P:(ic + 1) * P],
                                 wout_sb[k][:],
                                 start=(k == 0), stop=(k == KCH - 1))
            o = po.tile([P, COLS], F32, tag="ot")
            nc.vector.tensor_add(o[:], ps[:], bout_bc[:])
            nc.sync.dma_start(T["out_cols"][ic * P:(ic + 1) * P, :], o[:])
            ps2 = psD.tile([P, COLS], F32, tag="od", name=f"ctxp{ic}")
            for k in range(6):
                nc.tensor.matmul(ps2[:], w_sb[k][:, ic * P:(ic + 1) * P],
                                 cwout_sb[k][:],
                                 start=(k == 0), stop=(k == 5))
            cp_t = pu.tile([P, COLS], F32, tag=f"cp{ic}")
            nc.vector.tensor_add(cp_t[:], ps2[:], cbout_bc[:])
            ctx_part.append(cp_t)
        for ic in range(ICH):
            ps = psD.tile([P, COLS], F32, tag="od", name=f"ctxf{ic}")
            for k in (6, 7):
                nc.tensor.matmul(ps[:], w_sb[k][:, ic * P:(ic + 1) * P],
                                 cwout_sb[k][:],
                                 start=(k == 6), stop=(k == 7))
            o = po.tile([P, COLS], F32, tag="ot")
            nc.vector.tensor_add(o[:], ps[:], ctx_part[ic][:])
            nc.sync.dma_start(T["ctx_cols"][ic * P:(ic + 1) * P, :], o[:])
    stack.close()


def _get_nc():
    global _CACHED_NC
    if _CACHED_NC is None:
        _CACHED_NC = _build_nc()
    return _CACHED_NC


def _reorder_rows(w):
    """Reorder [INNER, :] rows to the uw_all K-chunk order (p-major, group X)."""
    chunks = []
    for p in range(4):
        for X in range(2):
            chunks.append(w[X * 512 + p * 128:X * 512 + (p + 1) * 128])
    return np.concatenate(chunks, axis=0)


def kernel(x, context, w_qk, w_v, cw_qk, cw_v, w_out, b_out, cw_out, cb_out):
    x = np.asarray(x, dtype=np.float32)
    context = np.asarray(context, dtype=np.float32)
    w_qk = np.asarray(w_qk, dtype=np.float32)
    w_v = np.asarray(w_v, dtype=np.float32)
    cw_qk = np.asarray(cw_qk, dtype=np.float32)
    cw_v = np.asarray(cw_v, dtype=np.float32)
    w_out_r = _reorder_rows(np.asarray(w_out, dtype=np.float32)).astype(ml_dtypes.bfloat16)
    cw_out_r = _reorder_rows(np.asarray(cw_out, dtype=np.float32)).astype(ml_dtypes.bfloat16)
    b_out = np.asarray(b_out, dtype=np.float32)
    cb_out = np.asarray(cb_out, dtype=np.float32)

    in_maps = []
    for c in range(8):
        b, g = c // 2, c % 2
        sl = slice(g * IL, (g + 1) * IL)
        in_maps.append({
            "xT": np.ascontiguousarray(x[b].T).astype(ml_dtypes.bfloat16),
            "ctxT": np.ascontiguousarray(context[b].T).astype(ml_dtypes.bfloat16),
            "wqk": np.ascontiguousarray(w_qk[:, sl]).astype(ml_dtypes.bfloat16),
            "wv": np.ascontiguousarray(w_v[:, sl]).astype(ml_dtypes.bfloat16),
            "cwqk": np.ascontiguousarray(cw_qk[:, sl]).astype(ml_dtypes.bfloat16),
            "cwv": np.ascontiguousarray(cw_v[:, sl]).astype(ml_dtypes.bfloat16),
            "wout": np.ascontiguousarray(w_out_r[:, sl]),
            "cwout": np.ascontiguousarray(cw_out_r[:, sl]),
            "bout": np.ascontiguousarray(b_out[None, sl]),
            "cbout": np.ascontiguousarray(cb_out[None, sl]),
        })

    nc = _get_nc()
    res = run_bass_kernel_spmd(nc, in_maps, list(range(8)))

    out = np.empty((B, N, DIM), dtype=np.float32)
    ctx_out = np.empty((B, N, DIM), dtype=np.float32)
    for b in range(B):
        out[b, :, 0:COLS] = res.results[2 * b]["out_cols"]
        out[b, :, COLS:] = res.results[2 * b + 1]["out_cols"]
        ctx_out[b, :, 0:COLS] = res.results[2 * b]["ctx_cols"]
        ctx_out[b, :, COLS:] = res.results[2 * b + 1]["ctx_cols"]
    return out, ctx_out


# revision 26
# speedup vs baseline: 1.0124x; 1.0124x over previous
"""Bidirectional cross-attention kernel for 8 Trainium2 NeuronCores.

Sharding: core c = 2*b + g handles batch b with head-group g (8 of 16 heads).
Each core projects Q/K/V/CV for its 8 heads (full sequence), computes both
softmax orientations of the shared similarity matrix, and forms the per-head
attention outputs U = attn @ cv and W = context_attn^T @ v (stored transposed,
pre-scaled by the softmax normalizers).  The two cores of a batch exchange
their U/W halves with a pairwise AllGather, after which each core computes a
disjoint 512-column slice of both final projections.  The host gather is pure
concatenation.

Layout notes:
 - x[b] / context[b] are fed pre-transposed ([dim, seq]) so every matmul sees
   its contraction dim on partitions; no on-device transposes are needed.
 - fp32 data is declared float32r so the PE runs at 1 cycle/row.
 - exp(sim) is stored in bf16; V/CV are stored in bf16 with an extra ones
   column per head so the PE produces the softmax normalizers for free.
 - w_out / cw_out are fed with rows pre-reordered to match the AllGather
   output layout, and column-sliced per core.
"""

import os
import sys

import numpy as np

for _p in ("/opt/trn_rl_repo", "/root/.axon_site/_ro/trn_rl_repo"):
    if os.path.isdir(_p) and _p not in sys.path:
        sys.path.append(_p)

import ml_dtypes  # noqa: E402
import concourse.bass as bass  # noqa: E402
import concourse.mybir as mybir  # noqa: E402
import concourse.tile as tile  # noqa: E402
from concourse import bacc  # noqa: E402
from concourse.bass_utils import run_bass_kernel_spmd  # noqa: E402

B, N, DIM = 4, 1024, 1024
H, DH = 16, 64
HL = 8            # heads per core
IL = HL * DH      # local inner width (512)
COLS = 512        # output columns per core
P = 128
PAIRS = HL // 2   # head pairs per core
KCH = DIM // P    # contraction chunks (8)
ICH = N // P      # sequence chunks (8)
SCALE = DH ** -0.5
GROUPS = [[0, 1], [2, 3], [4, 5], [6, 7]]

F32 = mybir.dt.float32
F32R = mybir.dt.float32r
BF16 = mybir.dt.bfloat16
EXP = mybir.ActivationFunctionType.Exp

_CACHED_NC = None


def _build_nc():
    nc = bacc.Bacc("TRN2", target_bir_lowering=False, debug=False, num_devices=8)

    xT = nc.dram_tensor("xT", [DIM, N], BF16, kind="ExternalInput")
    ctxT = nc.dram_tensor("ctxT", [DIM, N], BF16, kind="ExternalInput")
    wqk = nc.dram_tensor("wqk", [DIM, IL], BF16, kind="ExternalInput")
    wv = nc.dram_tensor("wv", [DIM, IL], BF16, kind="ExternalInput")
    cwqk = nc.dram_tensor("cwqk", [DIM, IL], BF16, kind="ExternalInput")
    cwv = nc.dram_tensor("cwv", [DIM, IL], BF16, kind="ExternalInput")
    wout = nc.dram_tensor("wout", [DIM, COLS], BF16, kind="ExternalInput")
    cwout = nc.dram_tensor("cwout", [DIM, COLS], BF16, kind="ExternalInput")
    bout = nc.dram_tensor("bout", [1, COLS], F32, kind="ExternalInput")
    cbout = nc.dram_tensor("cbout", [1, COLS], F32, kind="ExternalInput")

    out_cols = nc.dram_tensor("out_cols", [N, COLS], F32, kind="ExternalOutput")
    ctx_cols = nc.dram_tensor("ctx_cols", [N, COLS], F32, kind="ExternalOutput")

    with tile.TileContext(nc) as tc:
        # DRAM scratch (tracked by Tile)
        with tc.tile_pool(name="dram", bufs=1, space="DRAM") as dpool:
            uwl = [dpool.tile([256, N], BF16, tag=f"uwl{p}", name=f"uwl{p}")
                   for p in range(4)]
            uwa = [dpool.tile([512, N], BF16, tag=f"uwa{p}", name=f"uwa{p}")
                   for p in range(3)]
            uwa3u = dpool.tile([256, N], BF16, tag="uwa3u")
            uwa3w = dpool.tile([256, N], BF16, tag="uwa3w")
            normd = dpool.tile([16, N], F32, tag="normd")
            _build_body(nc, tc, dict(
                xT=xT, ctxT=ctxT, wqk=wqk, wv=wv, cwqk=cwqk, cwv=cwv,
                wout=wout, cwout=cwout, bout=bout, cbout=cbout,
                out_cols=out_cols, ctx_cols=ctx_cols,
                uwl=uwl, uwa=uwa, uwa3u=uwa3u, uwa3w=uwa3w, normd=normd,
            ))
    nc.compile()
    if os.environ.get("KERNEL_LDW_DEDUP", "1") == "1":
        _dedupe_ldweights(nc)
    return nc


def _dedupe_ldweights(nc):
    """Drop PE Ldweights that reload the exact weights already resident.

    bass emits one Ldweights per matmul; back-to-back matmuls that share a
    stationary operand (our N-half pairs) reload it redundantly, and the
    walrus pass that would elide these (--enable-ldw-opt) rejects this
    program.  The PE keeps its stationary operand across matmuls, so a
    repeat load with no attached semaphore activity can be removed.
    """
    def sig(i):
        a = i.ins[0]
        return (a.memref, a.offset, str(a.ap), str(a.dtype),
                str(i.tile_position), str(i.tile_size),
                str(i.perf_mode), str(i.is_transpose))

    removed = 0
    for fn in nc.m.functions:
        for bb in fn.blocks:
            last = None
            keep = []
            for i in bb.instructions:
                if isinstance(i, mybir.InstLdweights):
                    s = sig(i)
                    si = i.sync_info
                    if s == last and (si is None or
                                      (not si.on_wait and not si.on_update)):
                        removed += 1
                        continue
                    last = s
                elif isinstance(i, mybir.InstMatmult):
                    pass  # does not disturb the loaded weights
                elif getattr(i, "engine", None) == mybir.EngineType.PE:
                    last = None
                keep.append(i)
            if removed:
                bb.instructions = keep
    return removed


def _build_body(nc, tc, T):
    hs = 65  # head stride in the V/CV tiles (64 values + ones column)

    from contextlib import ExitStack
    stack = ExitStack()
    pqk = stack.enter_context(tc.tile_pool(name="pqk", bufs=1))
    pv = stack.enter_context(tc.tile_pool(name="pv", bufs=1))
    pf = stack.enter_context(tc.tile_pool(name="pf", bufs=1))
    with tc.tile_pool(name="pin", bufs=1) as pin, \
         tc.tile_pool(name="pw", bufs=12) as pw, \
         tc.tile_pool(name="psA", bufs=4, space="PSUM") as psA:

        # ---- Phase A: load transposed activations, project Q/K/V/CV ----
        def load_w(name, eng, tag):
            tiles = []
            for k in range(KCH):
                t = pw.tile([P, IL], BF16, tag=tag, name=f"{name}{k}")
                eng.dma_start(t[:], T[name][k * P:(k + 1) * P, :])
                tiles.append(t)
            return tiles

        # critical-path first: wqk + x on the sync queue, context side on the
        # scalar queue; output-projection weights are loaded at the end.
        xt = []
        ct = []
        t = pin.tile([P, N], BF16, tag="xT0")
        nc.sync.dma_start(t[:], T["xT"][0:P, :])
        xt.append(t)
        wqk_t = load_w("wqk", nc.sync, "w")
        for k in range(1, KCH):
            t = pin.tile([P, N], BF16, tag=f"xT{k}", name=f"xt{k}")
            nc.sync.dma_start(t[:], T["xT"][k * P:(k + 1) * P, :])
            xt.append(t)
        cwqk_t = load_w("cwqk", nc.scalar, "w2")
        for k in range(KCH):
            t = pin.tile([P, N], BF16, tag=f"cT{k}")
            nc.scalar.dma_start(t[:], T["ctxT"][k * P:(k + 1) * P, :])
            ct.append(t)

        def proj_T(src, wtiles, out_tag):
            """Per head-pair m: two zero-padded [128, N] tiles (head A rows
            0:64 / head B rows 64:128, rest zeros) so sim matmuls run with a
            full K=128 contraction (keeps the PE HAM-warm, enables FWL)."""
            outs = []
            pss = [psA.tile([P, N], F32, tag="pa", name=f"pt_{out_tag}{m}")
                   for m in range(IL // P)]
            for k in range(KCH):
                for m in range(IL // P):
                    lhsT = wtiles[k][:, m * P:(m + 1) * P]
                    nc.tensor.matmul(pss[m][:, 0:512], lhsT, src[k][:, 0:512],
                                     start=(k == 0), stop=(k == KCH - 1))
                    nc.tensor.matmul(pss[m][:, 512:1024], lhsT, src[k][:, 512:1024],
                                     start=(k == 0), stop=(k == KCH - 1))
            for m in range(IL // P):
                ps = pss[m]
                pa = pqk.tile([P, N], BF16, tag=f"{out_tag}a{m}")
                nc.vector.tensor_copy(pa[0:DH, :], ps[0:DH, :])
                nc.vector.memset(pa[DH:P, :], 0.0)
                pb = pqk.tile([P, N], BF16, tag=f"{out_tag}b{m}")
                nc.vector.memset(pb[0:DH, :], 0.0)
                nc.vector.tensor_copy(pb[DH:P, :], ps[DH:P, :])
                outs.append((pa, pb))
            return outs

        def proj_V(src, wtiles, out_tag):
            """out[ic] [128, 8*65] bf16 = src-rows @ w, head-strided + ones."""
            outs = []
            for ic in range(ICH):
                psf = psA.tile([P, N], F32, tag="pa", name=f"pav_{out_tag}{ic}")
                ps = psf[:, 0:IL]
                for k in range(KCH):
                    nc.tensor.matmul(ps[:], src[k][:, ic * P:(ic + 1) * P],
                                     wtiles[k][:],
                                     start=(k == 0), stop=(k == KCH - 1))
                o = pv.tile([P, HL * hs], BF16, tag=f"{out_tag}{ic}")
                dst = o[:].rearrange("p (h e) -> p h e", e=hs)
                nc.vector.tensor_copy(dst[:, :, 0:DH],
                                      ps[:].rearrange("p (h e) -> p h e", e=DH))
                nc.vector.memset(dst[:, :, DH:hs], 1.0)
                outs.append(o)
            return outs

        QT = proj_T(xt, wqk_t, "qt")
        wv_t = load_w("wv", nc.sync, "w")
        KT = proj_T(ct, cwqk_t, "kt")
        cwv_t = load_w("cwv", nc.scalar, "w2")
        V = proj_V(xt, wv_t, "v")
        CV = proj_V(ct, cwv_t, "cv")

        # output-side weights/biases (needed only in the final phase)
        bout_bc = pf.tile([P, COLS], F32, tag="bb")
        nc.scalar.dma_start(bout_bc[:], T["bout"][:].to_broadcast((P, COLS)))
        cbout_bc = pf.tile([P, COLS], F32, tag="cbb")
        nc.scalar.dma_start(cbout_bc[:], T["cbout"][:].to_broadcast((P, COLS)))
        wout_sb = []
        cwout_sb = []
        for k in range(KCH):
            t = pf.tile([P, COLS], BF16, tag=f"wo{k}")
            nc.scalar.dma_start(t[:], T["wout"][k * P:(k + 1) * P, :])
            wout_sb.append(t)
            t = pf.tile([P, COLS], BF16, tag=f"cwo{k}")
            nc.scalar.dma_start(t[:], T["cwout"][k * P:(k + 1) * P, :])
            cwout_sb.append(t)

    # ---- Phase B: per head-pair attention ----
    pu = stack.enter_context(tc.tile_pool(name="pu", bufs=1))
    u_sb = [None] * KCH
    w_sb = [None] * KCH

    with tc.tile_pool(name="pe", bufs=10) as pe, \
         tc.tile_pool(name="pn", bufs=2) as pn, \
         tc.tile_pool(name="psB", bufs=2, space="PSUM") as psB:

        def load_uw(k, src_tile, u_off, w_off):
            usrc = src_tile if src_tile is not None else T["uwa3u"]
            wsrc = src_tile if src_tile is not None else T["uwa3w"]
            t = pu.tile([P, N], BF16, tag=f"ua{k}")
            nc.sync.dma_start(t[:], usrc[u_off:u_off + P, :])
            u_sb[k] = t
            t = pu.tile([P, N], BF16, tag=f"wa{k}")
            nc.sync.dma_start(t[:], wsrc[w_off:w_off + P, :])
            w_sb[k] = t

        for p in range(PAIRS):
            E = [[None] * ICH, [None] * ICH]
            ET = [[None] * ICH, [None] * ICH]

            def norm_store(psum, slot, dst, dst_row):
                """scale rows 0:64 of psum by 1/row64 (per free element), store."""
                rst = pn.tile([DH + 1, N], F32, tag="rst")
                nc.vector.tensor_copy(rst[:], psum[0:DH + 1, :])
                nc.sync.dma_start(T["normd"][slot:slot + 1, :], rst[DH:DH + 1, :])
                rbc = pn.tile([DH, N], F32, tag="rbc")
                nc.sync.dma_start(
                    rbc[:], T["normd"][slot:slot + 1, :].to_broadcast((DH, N)))
                nc.vector.reciprocal_approx_fast(rbc[:], rbc[:])
                ubf = pn.tile([DH, N], BF16, tag="ubf")
                nc.vector.tensor_mul(ubf[:], rst[0:DH, :], rbc[:])
                nc.sync.dma_start(dst[dst_row:dst_row + DH, :], ubf[:])

            # --- simT -> ET, with U-accumulation laddered in (lag 2) ---
            ups = [psB.tile([P, N], F32, tag="uw", name=f"ups{p}_{hh}")
                   for hh in range(2)]

            def u_step(hh, jc):
                h = 2 * p + hh
                lhsT = CV[jc][:, h * hs:(h + 1) * hs]
                nc.tensor.matmul(ups[hh][0:hs, 0:512], lhsT, ET[hh][jc][:, 0:512],
                                 start=(jc == 0), stop=(jc == ICH - 1))
                nc.tensor.matmul(ups[hh][0:hs, 512:1024], lhsT,
                                 ET[hh][jc][:, 512:1024],
                                 start=(jc == 0), stop=(jc == ICH - 1))

            for jc in range(ICH):
                for hh in range(2):
                    part = slice(hh * DH, (hh + 1) * DH)
                    ps = psB.tile([P, N], F32, tag="sim")
                    lhsT = KT[p][hh][part, jc * P:(jc + 1) * P]
                    nc.tensor.matmul(ps[:, 0:512], lhsT, QT[p][hh][part, 0:512],
                                     start=True, stop=True)
                    nc.tensor.matmul(ps[:, 512:1024], lhsT, QT[p][hh][part, 512:1024],
                                     start=True, stop=True)
                    e = pe.tile([P, N], BF16, tag="ET")
                    nc.scalar.activation(e[:], ps[:], EXP, scale=SCALE)
                    ET[hh][jc] = e
                if jc >= 2:
                    for hh in range(2):
                        u_step(hh, jc - 2)
            for jc in (ICH - 2, ICH - 1):
                for hh in range(2):
                    u_step(hh, jc)
            for hh in range(2):
                norm_store(ups[hh], p * 4 + hh, T["uwl"][p], hh * DH)

            if p == 3:
                nc.gpsimd.collective_compute(
                    "AllGather", mybir.AluOpType.bypass,
                    replica_groups=GROUPS,
                    ins=[T["uwl"][3][0:128, :]],
                    outs=[T["uwa3u"][:]],
                )

            # --- sim -> E, with W-accumulation laddered in (lag 2) ---
            wps = [psB.tile([P, N], F32, tag="uw", name=f"wps{p}_{hh}")
                   for hh in range(2)]

            def w_step(hh, ic):
                h = 2 * p + hh
                lhsT = V[ic][:, h * hs:(h + 1) * hs]
                nc.tensor.matmul(wps[hh][0:hs, 0:512], lhsT, E[hh][ic][:, 0:512],
                                 start=(ic == 0), stop=(ic == ICH - 1))
                nc.tensor.matmul(wps[hh][0:hs, 512:1024], lhsT,
                                 E[hh][ic][:, 512:1024],
                                 start=(ic == 0), stop=(ic == ICH - 1))

            for ic in range(ICH):
                for hh in range(2):
                    part = slice(hh * DH, (hh + 1) * DH)
                    ps = psB.tile([P, N], F32, tag="sim")
                    lhsT = QT[p][hh][part, ic * P:(ic + 1) * P]
                    nc.tensor.matmul(ps[:, 0:512], lhsT, KT[p][hh][part, 0:512],
                                     start=True, stop=True)
                    nc.tensor.matmul(ps[:, 512:1024], lhsT, KT[p][hh][part, 512:1024],
                                     start=True, stop=True)
                    e = pe.tile([P, N], BF16, tag="E")
                    nc.scalar.activation(e[:], ps[:], EXP, scale=SCALE)
                    E[hh][ic] = e
                if ic >= 2:
                    for hh in range(2):
                        w_step(hh, ic - 2)
            for ic in (ICH - 2, ICH - 1):
                for hh in range(2):
                    w_step(hh, ic)
            for hh in range(2):
                norm_store(wps[hh], p * 4 + 2 + hh, T["uwl"][p], 128 + hh * DH)

            # exchange this pair's U/W halves within the batch pair.
            # Pair 3 is split so its U half can ship before W finishes.
            if p < 3:
                nc.gpsimd.collective_compute(
                    "AllGather", mybir.AluOpType.bypass,
                    replica_groups=GROUPS,
                    ins=[T["uwl"][p][:]],
                    outs=[T["uwa"][p][:]],
                )
                load_uw(2 * p, T["uwa"][p], 0, 128)
                load_uw(2 * p + 1, T["uwa"][p], 256, 384)
            else:
                nc.gpsimd.collective_compute(
                    "AllGather", mybir.AluOpType.bypass,
                    replica_groups=GROUPS,
                    ins=[T["uwl"][3][128:256, :]],
                    outs=[T["uwa3w"][:]],
                )
                load_uw(6, None, 0, 0)
                load_uw(7, None, 128, 128)

    # ---- Phase C/D: load gathered U/W, final projections ----
    with tc.tile_pool(name="po", bufs=3) as po, \
         tc.tile_pool(name="psD", bufs=6, space="PSUM") as psD:

        # out-projection (needs all U chunks; U arrives before W), with the
        # ctx-projection's early chunks (pairs 0-2) interleaved so the PE has
        # work while pair 3's W AllGather is still in flight.
        ctx_part = []
        for ic in range(ICH):
            ps = psD.tile([P, COLS], F32, tag="od")
            for k in range(KCH):
                nc.tensor.matmul(ps[:], u_sb[k][:, ic * P:(ic + 1) * P],
                                 wout_sb[k][:],
                                 start=(k == 0), stop=(k == KCH - 1))
            o = po.tile([P, COLS], F32, tag="ot")
            nc.vector.tensor_add(o[:], ps[:], bout_bc[:])
            nc.sync.dma_start(T["out_cols"][ic * P:(ic + 1) * P, :], o[:])
            ps2 = psD.tile([P, COLS], F32, tag="od", name=f"ctxp{ic}")
            for k in range(6):
                nc.tensor.matmul(ps2[:], w_sb[k][:, ic * P:(ic + 1) * P],
                                 cwout_sb[k][:],
                                 start=(k == 0), stop=(k == 5))
            cp_t = pu.tile([P, COLS], F32, tag=f"cp{ic}")
            nc.vector.tensor_add(cp_t[:], ps2[:], cbout_bc[:])
            ctx_part.append(cp_t)
        for ic in range(ICH):
            ps = psD.tile([P, COLS], F32, tag="od", name=f"ctxf{ic}")
            for k in (6, 7):
                nc.tensor.matmul(ps[:], w_sb[k][:, ic * P:(ic + 1) * P],
                                 cwout_sb[k][:],
                                 start=(k == 6), stop=(k == 7))
            o = po.tile([P, COLS], F32, tag="ot")
            nc.vector.tensor_add(o[:], ps[:], ctx_part[ic][:])
            nc.sync.dma_start(T["ctx_cols"][ic * P:(ic + 1) * P, :], o[:])
    stack.close()


def _get_nc():
    global _CACHED_NC
    if _CACHED_NC is None:
        _CACHED_NC = _build_nc()
    return _CACHED_NC


def _reorder_rows(w):
    """Reorder [INNER, :] rows to the uw_all K-chunk order (p-major, group X)."""
    chunks = []
    for p in range(4):
        for X in range(2):
            chunks.append(w[X * 512 + p * 128:X * 512 + (p + 1) * 128])
    return np.concatenate(chunks, axis=0)


def kernel(x, context, w_qk, w_v, cw_qk, cw_v, w_out, b_out, cw_out, cb_out):
    x = np.asarray(x, dtype=np.float32)
    context = np.asarray(context, dtype=np.float32)
    w_qk = np.asarray(w_qk, dtype=np.float32)
    w_v = np.asarray(w_v, dtype=np.float32)
    cw_qk = np.asarray(cw_qk, dtype=np.float32)
    cw_v = np.asarray(cw_v, dtype=np.float32)
    w_out_r = _reorder_rows(np.asarray(w_out, dtype=np.float32)).astype(ml_dtypes.bfloat16)
    cw_out_r = _reorder_rows(np.asarray(cw_out, dtype=np.float32)).astype(ml_dtypes.bfloat16)
    b_out = np.asarray(b_out, dtype=np.float32)
    cb_out = np.asarray(cb_out, dtype=np.float32)

    in_maps = []
    for c in range(8):
        b, g = c // 2, c % 2
        sl = slice(g * IL, (g + 1) * IL)
        in_maps.append({
            "xT": np.ascontiguousarray(x[b].T).astype(ml_dtypes.bfloat16),
            "ctxT": np.ascontiguousarray(context[b].T).astype(ml_dtypes.bfloat16),
            "wqk": np.ascontiguousarray(w_qk[:, sl]).astype(ml_dtypes.bfloat16),
            "wv": np.ascontiguousarray(w_v[:, sl]).astype(ml_dtypes.bfloat16),
            "cwqk": np.ascontiguousarray(cw_qk[:, sl]).astype(ml_dtypes.bfloat16),
            "cwv": np.ascontiguousarray(cw_v[:, sl]).astype(ml_dtypes.bfloat16),
            "wout": np.ascontiguousarray(w_out_r[:, sl]),
            "cwout": np.ascontiguousarray(cw_out_r[:, sl]),
            "bout": np.ascontiguousarray(b_out[None, sl]),
            "cbout": np.ascontiguousarray(cb_out[None, sl]),
        })

    nc = _get_nc()
    res = run_bass_kernel_spmd(nc, in_maps, list(range(8)))

    out = np.empty((B, N, DIM), dtype=np.float32)
    ctx_out = np.empty((B, N, DIM), dtype=np.float32)
    for b in range(B):
        out[b, :, 0:COLS] = res.results[2 * b]["out_cols"]
        out[b, :, COLS:] = res.results[2 * b + 1]["out_cols"]
        ctx_out[b, :, 0:COLS] = res.results[2 * b]["ctx_cols"]
        ctx_out[b, :, COLS:] = res.results[2 * b + 1]["ctx_cols"]
    return out, ctx_out
